# revision 1
# baseline (speedup 1.0000x reference)
"""GAT with autoencoder + residuals on 8 trn2 NeuronCores (Bass/Tile).

Strategy: nodes are renumbered by in-degree (desc) and dealt round-robin to the
8 cores; each dst-node owns one SBUF partition of its 128-node tile, and its
incoming edges occupy chunk columns of that partition. Gathers of source
features use dma_gather (int16 indices -> lo/hi split tables), split across
the 4 SWDGE queues to avoid ring backpressure. Attention weights are
normalized (alpha = e/sum e) BEFORE aggregation; aggregation runs
data-stationary on the tensor engine (lhsT = gathered chunk, rhs = identity),
accumulating feature-major directly on top of the residual-projection PSUM.
All dense math in bf16. Small weights are replicated; two AllGathers publish
the per-core projection tables between phases.
"""
import hashlib
import numpy as np
import ml_dtypes

import concourse.bacc as bacc
import concourse.mybir as mybir
import concourse.tile as tile
from concourse import bass_utils

# model sizes (fixed by the problem)
N = 50000
IN = 512
ENC = 256
HID = 32
HEADS = 4
OUT = 40
SLOPE = 0.2

NCORE = 8
P = 128
NTILE = 49
NPC = NTILE * P          # 6272 nodes per core
NPAD = NCORE * NPC       # 50176
LO = 32768               # lo table rows [0, LO)
HIOFF = NPAD - 32768     # hi table rows [HIOFF, NPAD)
# chunk-major table layout: tables are built in CHUNKS of tile ranges so the
# AllGather of chunk k can launch as soon as its producer groups finish.
CH_T = [0, 16, 28, 40, 49]           # tile boundaries of the 4 chunks
CH_R0 = [t * P for t in CH_T]        # per-core row boundaries
CH_CR = [CH_R0[i + 1] - CH_R0[i] for i in range(4)]   # rows per core per chunk
CH_CB = [0]
for i in range(4):
    CH_CB.append(CH_CB[-1] + NCORE * CH_CR[i])        # table base of chunk

F32 = mybir.dt.float32
BF16 = mybir.dt.bfloat16
I16 = mybir.dt.int16
AF = mybir.ActivationFunctionType
ALU = mybir.AluOpType
AX = mybir.AxisListType

TB1 = 256   # bf16 columns per table1 row: [h(128 bf16) | a_src(4 f32) | pad]
TB2 = 128   # bf16 columns per table2 row: [z(40 bf16) | b_src(1 bf16) | pad]
SUBCH = 6   # max chunks per sub-gather

_cache = {}


def _wrap_idx(blk):
    """[128, NB] slot-major block -> dma_gather idx layout [128, 8*NB] int16."""
    nb = blk.shape[1]
    flat = blk.T.reshape(-1)                 # j = c*128 + p
    w = flat.reshape(-1, 16).T               # [16, 8*NB]
    return np.tile(w, (8, 1)).astype(np.int16)


def _prepare(inputs):
    x = np.asarray(inputs["x"], np.float32)
    ei = np.asarray(inputs["edge_index"]).astype(np.int64)
    src = np.concatenate([ei[0], np.arange(N, dtype=np.int64)])
    dst = np.concatenate([ei[1], np.arange(N, dtype=np.int64)])

    deg = np.bincount(dst, minlength=NPAD)
    order = np.argsort(-deg, kind="stable")
    rank = np.empty(NPAD, np.int64)
    rank[order] = np.arange(NPAD)
    core_of = rank % NCORE
    pos_of = rank // NCORE
    tidx_of = core_of * NPC + pos_of         # table row of each (old) node

    er = rank[dst]
    est = tidx_of[src]
    lo_ex = est < HIOFF
    hi_ex = est >= LO
    key2 = np.where(lo_ex, 0, np.where(hi_ex, 2, 1))

    nlo = np.bincount(er[lo_ex], minlength=NPAD)
    nhi = np.bincount(er[hi_ex], minlength=NPAD)
    degr = deg[order]

    NLO = np.zeros(NTILE, np.int64)
    NHI = np.zeros(NTILE, np.int64)
    l_of = np.zeros(NPAD, np.int64)
    B = NCORE * P
    for t in range(NTILE):
        blk = slice(t * B, (t + 1) * B)
        NLO[t] = nlo[blk].max()
        l = np.minimum(degr[blk] - nhi[blk], NLO[t])
        l_of[blk] = l
        NHI[t] = max(nhi[blk].max(), (degr[blk] - l).max(), 0)
    NC = NLO + NHI
    CSTART = np.zeros(NTILE, np.int64)
    CSTART[1:] = np.cumsum(NC)[:-1]
    CTOT = int(NC.sum())

    # per-edge slot assignment
    eord = np.lexsort((key2, er))
    er_s = er[eord]
    est_s = est[eord]
    boundaries = np.flatnonzero(np.r_[True, er_s[1:] != er_s[:-1]])
    counts = np.diff(np.r_[boundaries, er_s.size])
    j = np.arange(er_s.size) - np.repeat(boundaries, counts)
    lcap = l_of[er_s]
    side_lo = j < lcap
    et_s = (er_s // NCORE) // P
    epart_s = (er_s // NCORE) % P
    ecore_s = er_s % NCORE
    col = np.where(side_lo, j, NLO[et_s] + (j - lcap))
    val = np.where(side_lo, est_s, est_s - HIOFF).astype(np.int16)

    sval = np.zeros((NCORE, P, CTOT), np.int16)
    mask = np.zeros((NCORE, P, CTOT), np.float32)
    colg = CSTART[et_s] + col
    sval[ecore_s, epart_s, colg] = val
    mask[ecore_s, epart_s, colg] = 1.0

    # wrapped idx blocks per (tile, side) concatenated; WSTART per gather
    WSTART = []
    w_off = 0
    for t in range(NTILE):
        WSTART.append((w_off, w_off + 8 * int(NLO[t])))
        w_off += 8 * int(NLO[t]) + 8 * int(NHI[t])
    WTOT = w_off
    idx_all = np.zeros((NCORE, P, WTOT), np.int16)
    for c in range(NCORE):
        for t in range(NTILE):
            cs = CSTART[t]
            lo_w, hi_w = WSTART[t]
            if NLO[t]:
                idx_all[c][:, lo_w:lo_w + 8 * int(NLO[t])] = _wrap_idx(
                    sval[c][:, cs:cs + int(NLO[t])])
            if NHI[t]:
                idx_all[c][:, hi_w:hi_w + 8 * int(NHI[t])] = _wrap_idx(
                    sval[c][:, cs + int(NLO[t]):cs + int(NC[t])])

    # per-core x (transposed, feature-major, bf16)
    xp = np.zeros((NPAD, IN), np.float32)
    xp[:N] = x
    old_ids = np.empty((NCORE, NPC), np.int64)
    x_t = np.empty((NCORE, IN, NPC), ml_dtypes.bfloat16)
    for c in range(NCORE):
        ids = order[c + NCORE * np.arange(NPC)]
        old_ids[c] = ids
        x_t[c] = xp[ids].T.astype(ml_dtypes.bfloat16)

    # replicated derived weights
    w = lambda k: np.asarray(inputs[k], np.float32)
    bf = lambda a: np.ascontiguousarray(a).astype(ml_dtypes.bfloat16)
    a1sd = np.zeros((P, 8), np.float32)
    for h in range(HEADS):
        a1sd[h * HID:(h + 1) * HID, h] = w("gat1_att_src")[h]
        a1sd[h * HID:(h + 1) * HID, 4 + h] = w("gat1_att_dst")[h]
    vs2 = w("gat2_w") @ w("gat2_att_src")[0]
    vd2 = w("gat2_w") @ w("gat2_att_dst")[0]
    lhsT2 = np.concatenate([w("gat2_w"), vs2[:, None], vd2[:, None]], 1)  # [128,42]
    bias12 = (w("gat1_b") + w("res1_b")).reshape(P, 1)
    b2c = np.zeros((P, 1), np.float32)
    b2c[:OUT, 0] = w("gat2_b") + w("res2_b")

    consts = {
        "identb": np.eye(P, dtype=np.float32).astype(ml_dtypes.bfloat16),
        "ident": np.eye(P, dtype=np.float32),
        "w1": bf(w("ae_w1")), "b1p": w("ae_b1").reshape(2, P).T.copy(),
        "w2": bf(w("ae_w2")), "b2p": w("ae_b2").reshape(2, P).T.copy(),
        "g1w": bf(w("gat1_w")), "a1sd": bf(a1sd),
        "res1w": bf(w("res1_w")), "b12p": bias12,
        "l2p": bf(lhsT2), "r2w": bf(w("res2_w")), "b2cp": b2c,
    }
    meta = {
        "NLO": NLO.tolist(), "NHI": NHI.tolist(),
        "CSTART": CSTART.tolist(), "CTOT": CTOT,
        "WSTART": WSTART, "WTOT": WTOT,
        "key": "v2:" + hashlib.sha1(ei.tobytes()).hexdigest(),
    }
    in_maps = []
    for c in range(NCORE):
        m = {"x_t": x_t[c], "idx_all": idx_all[c], "mask_all": mask[c]}
        m.update(consts)
        in_maps.append(m)
    return meta, in_maps, old_ids


def _build(meta):
    NLO, NHI = meta["NLO"], meta["NHI"]
    CSTART, CTOT = meta["CSTART"], meta["CTOT"]
    WSTART, WTOT = meta["WSTART"], meta["WTOT"]

    nc = bacc.Bacc("TRN2", target_bir_lowering=False, debug=False,
                   num_devices=NCORE, num_swdge_queues=4)
    # I/O
    x_t = nc.dram_tensor("x_t", [IN, NPC], BF16, kind="ExternalInput")
    idx_all = nc.dram_tensor("idx_all", [P, WTOT], I16, kind="ExternalInput")
    mask_all = nc.dram_tensor("mask_all", [P, CTOT], F32, kind="ExternalInput")
    identb = nc.dram_tensor("identb", [P, P], BF16, kind="ExternalInput")
    ident = nc.dram_tensor("ident", [P, P], F32, kind="ExternalInput")
    w1 = nc.dram_tensor("w1", [IN, ENC], BF16, kind="ExternalInput")
    b1p = nc.dram_tensor("b1p", [P, 2], F32, kind="ExternalInput")
    w2 = nc.dram_tensor("w2", [ENC, ENC], BF16, kind="ExternalInput")
    b2p = nc.dram_tensor("b2p", [P, 2], F32, kind="ExternalInput")
    g1w = nc.dram_tensor("g1w", [ENC, P], BF16, kind="ExternalInput")
    a1sd = nc.dram_tensor("a1sd", [P, 8], BF16, kind="ExternalInput")
    res1w = nc.dram_tensor("res1w", [ENC, P], BF16, kind="ExternalInput")
    b12p = nc.dram_tensor("b12p", [P, 1], F32, kind="ExternalInput")
    l2p = nc.dram_tensor("l2p", [P, 42], BF16, kind="ExternalInput")
    r2w = nc.dram_tensor("r2w", [P, OUT], BF16, kind="ExternalInput")
    b2cp = nc.dram_tensor("b2cp", [P, 1], F32, kind="ExternalInput")
    out_d = nc.dram_tensor("out", [OUT, NPC], F32, kind="ExternalOutput")

    groups = [(gi, min(4, NTILE - gi * 4)) for gi in range((NTILE + 3) // 4)]

    with tile.TileContext(nc) as tc:
        with (
            tc.tile_pool(name="const", bufs=1) as cp,
            tc.tile_pool(name="pers", bufs=1) as pp,
            tc.tile_pool(name="dram", bufs=1, space="DRAM") as dp,
            tc.tile_pool(name="xk", bufs=2) as xkp,
            tc.tile_pool(name="wk", bufs=2) as wk,
            tc.tile_pool(name="tb", bufs=3) as tbp,
            tc.tile_pool(name="gg", bufs=2) as gp,
            tc.tile_pool(name="gg2", bufs=3) as gp2,
            tc.tile_pool(name="pbig", bufs=2, space="PSUM") as pb,
            tc.tile_pool(name="pzo", bufs=2, space="PSUM") as pz,
            tc.tile_pool(name="ptr", bufs=4, space="PSUM") as ptp,
        ):
            # ---- load constants
            identb_sb = cp.tile([P, P], BF16)
            nc.sync.dma_start(identb_sb[:], identb[:])
            ident_sb = cp.tile([P, P], F32)
            nc.sync.dma_start(ident_sb[:], ident[:])
            w1_sb = cp.tile([P, 4 * ENC], BF16)
            nc.sync.dma_start(
                w1_sb[:].rearrange("p (k m) -> p k m", m=ENC),
                w1[:].rearrange("(k p) m -> p k m", p=P))
            w2_sb = cp.tile([P, 2 * ENC], BF16)
            nc.sync.dma_start(
                w2_sb[:].rearrange("p (k m) -> p k m", m=ENC),
                w2[:].rearrange("(k p) m -> p k m", p=P))
            g1w_sb = cp.tile([P, 2 * P], BF16)
            nc.sync.dma_start(
                g1w_sb[:].rearrange("p (k m) -> p k m", m=P),
                g1w[:].rearrange("(k p) m -> p k m", p=P))
            res1w_sb = cp.tile([P, 2 * P], BF16)
            nc.sync.dma_start(
                res1w_sb[:].rearrange("p (k m) -> p k m", m=P),
                res1w[:].rearrange("(k p) m -> p k m", p=P))
            b1_sb = cp.tile([P, 2], F32)
            nc.sync.dma_start(b1_sb[:], b1p[:])
            b2_sb = cp.tile([P, 2], F32)
            nc.sync.dma_start(b2_sb[:], b2p[:])
            a1sd_sb = cp.tile([P, 8], BF16)
            nc.sync.dma_start(a1sd_sb[:], a1sd[:])
            b12_sb = cp.tile([P, 1], F32)
            nc.sync.dma_start(b12_sb[:], b12p[:])
            l2_sb = cp.tile([P, 42], BF16)
            nc.sync.dma_start(l2_sb[:], l2p[:])
            r2w_sb = cp.tile([P, OUT], BF16)
            nc.sync.dma_start(r2w_sb[:], r2w[:])
            b2c_sb = cp.tile([P, 1], F32)
            nc.sync.dma_start(b2c_sb[:], b2cp[:])
            idx_sb = pp.tile([P, WTOT], I16)
            nc.sync.dma_start(idx_sb[:], idx_all[:])
            mask_sb = pp.tile([P, CTOT], F32)
            nc.sync.dma_start(mask_sb[:], mask_all[:])

            adst_nm = pp.tile([P, NTILE * 4], F32)
            bdst_nm = pp.tile([P, NTILE], F32)
            xe_res = pp.tile([P, 2 * NPC], BF16)       # encoder output, resident
            res2F = pp.tile([OUT, NPC], F32)           # res2 branch, feature-major

            # internal DRAM
            slice1 = dp.tile([NPC, TB1], BF16)
            full1 = dp.tile([NPAD, TB1], BF16, addr_space="Shared")
            slice2 = dp.tile([NPC, TB2], BF16)
            full2 = dp.tile([NPAD, TB2], BF16, addr_space="Shared")

            # ---- phase A+B: autoencoder, GAT1 projections, table1 rows
            for gi, gn in groups:
                GW = gn * P
                g0 = gi * 4 * P
                xks = []
                for k in range(4):
                    xk = xkp.tile([P, GW], BF16, tag=f"xk{k}")
                    nc.sync.dma_start(xk[:], x_t[k * P:(k + 1) * P, g0:g0 + GW])
                    xks.append(xk)
                z1s = []
                for m in range(2):
                    ps1 = pb.tile([P, GW], F32, tag="pbig")
                    for k in range(4):
                        nc.tensor.matmul(
                            out=ps1[:], lhsT=w1_sb[:, k * ENC + m * P:k * ENC + (m + 1) * P],
                            rhs=xks[k][:], start=(k == 0), stop=(k == 3))
                    z1 = wk.tile([P, GW], BF16, tag=f"z1{m}")
                    nc.scalar.activation(z1[:], ps1[:], AF.Relu, bias=b1_sb[:, m:m + 1])
                    z1s.append(z1)
                for m in range(2):
                    ps2 = pb.tile([P, GW], F32, tag="pbig")
                    for k in range(2):
                        nc.tensor.matmul(
                            out=ps2[:], lhsT=w2_sb[:, k * ENC + m * P:k * ENC + (m + 1) * P],
                            rhs=z1s[k][:], start=(k == 0), stop=(k == 1))
                    nc.scalar.activation(xe_res[:, m * NPC + g0:m * NPC + g0 + GW],
                                         ps2[:], AF.Relu, bias=b2_sb[:, m:m + 1])
                # h = xe @ gat1_w ; a_src/a_dst
                psh = pb.tile([P, GW], F32, tag="pbig")
                for k in range(2):
                    nc.tensor.matmul(out=psh[:], lhsT=g1w_sb[:, k * P:(k + 1) * P],
                                     rhs=xe_res[:, k * NPC + g0:k * NPC + g0 + GW],
                                     start=(k == 0), stop=(k == 1))
                h_sb = wk.tile([P, GW], BF16, tag="hsb")
                nc.vector.tensor_copy(h_sb[:], psh[:])
                psa = pz.tile([8, GW], F32, tag="pzo")
                nc.tensor.matmul(out=psa[:], lhsT=a1sd_sb[:], rhs=h_sb[:],
                                 start=True, stop=True)
                a_sd = wk.tile([8, GW], F32, tag="asd")
                nc.vector.tensor_copy(a_sd[:], psa[:])
                for s in range(gn):
                    ti = gi * 4 + s
                    ptr1 = ptp.tile([P, P], BF16, tag="ptr")
                    nc.tensor.transpose(ptr1[:], h_sb[:, s * P:(s + 1) * P], identb_sb[:])
                    tb1 = tbp.tile([P, TB1], BF16, tag="tb1")
                    nc.vector.tensor_copy(tb1[:, 0:P], ptr1[:])
                    ptr2 = ptp.tile([P, 8], F32, tag="ptr")
                    nc.tensor.transpose(ptr2[:], a_sd[:, s * P:(s + 1) * P], ident_sb[0:8, 0:8])
                    f32v = tb1[:].bitcast(F32)
                    nc.vector.tensor_copy(f32v[:, 64:68], ptr2[:, 0:4])
                    nc.vector.tensor_copy(adst_nm[:, ti * 4:(ti + 1) * 4], ptr2[:, 4:8])
                    nc.sync.dma_start(slice1[ti * P:(ti + 1) * P, :], tb1[:])

            # ---- AllGather table1
            nc.gpsimd.collective_compute(
                "AllGather", ALU.bypass, replica_groups=[list(range(NCORE))],
                ins=[slice1[:]], outs=[full1[:]])

            qrot = [0]

            def split_gather(G, ti, tb, full):
                """4-queue-split gathers of the tile's lo/hi chunk runs."""
                nlo, nhi = NLO[ti], NHI[ti]
                lo_w, hi_w = WSTART[ti]
                Gv = G[:].rearrange("p (c e) -> p c e", e=tb)
                for side, n0, wb, c0, t0, t1 in (
                        (0, nlo, lo_w, 0, 0, LO),
                        (1, nhi, hi_w, nlo, HIOFF, NPAD)):
                    a = 0
                    while a < n0:
                        b = min(a + SUBCH, n0)
                        nb = b - a
                        nc.gpsimd.dma_gather(
                            Gv[:, c0 + a:c0 + b, :],
                            full[t0:t1, :], idx_sb[:, wb + 8 * a:wb + 8 * b],
                            128 * nb, 128 * nb, tb,
                            queue_num=qrot[0] % 4, single_packet=True)
                        qrot[0] += 1
                        a = b

            def gat1_tile(ti, psr, s):
                """Gather + attention + aggregation for one dst tile; the
                aggregation accumulates feature-major into psr[:, s*P:(s+1)*P]
                on top of the res1 projection already there."""
                nlo, nhi = NLO[ti], NHI[ti]
                ncc = nlo + nhi
                G = gp.tile([P, ncc * TB1], BF16, tag="G1")
                split_gather(G, ti, TB1, full1)
                G3 = G[:].rearrange("p (c e) -> p c e", e=TB1)
                Gf = G[:].bitcast(F32).rearrange("p (c f) -> p c f", f=P)
                # e = a_src + a_dst  (head-major [p, 4, ncc])
                ebuf = wk.tile([P, ncc * 4], F32, tag="ebuf")
                eb_h = ebuf[:].rearrange("p (f c) -> p f c", f=4)
                nc.vector.tensor_tensor(
                    out=eb_h, in0=Gf[:, :, 64:68].rearrange("p c f -> p f c"),
                    in1=adst_nm[:, ti * 4:(ti + 1) * 4].to_broadcast([P, 4, ncc]),
                    op=ALU.add)
                etmp = wk.tile([P, ncc * 4], F32, tag="etmp")
                nc.vector.tensor_scalar_mul(etmp[:], ebuf[:], SLOPE)
                nc.vector.tensor_tensor(out=ebuf[:], in0=ebuf[:], in1=etmp[:],
                                        op=ALU.max)
                nc.scalar.activation(ebuf[:], ebuf[:], AF.Exp)
                # mask (c-major view of head-major buffer)
                eb_c = ebuf[:].rearrange("p (f c) -> p c f", c=ncc)
                msl = mask_sb[:, CSTART[ti]:CSTART[ti] + ncc]
                nc.vector.tensor_tensor(out=eb_c, in0=eb_c,
                                        in1=msl.to_broadcast([P, ncc, 4]),
                                        op=ALU.mult)
                # alpha = e / sum_c e   (normalize before aggregation)
                dsum = wk.tile([P, 4], F32, tag="dsum")
                nc.vector.tensor_reduce(dsum[:], eb_h, AX.X, ALU.add)
                nc.vector.tensor_scalar_max(dsum[:], dsum[:], 1e-16)
                rec = wk.tile([P, 4], F32, tag="rec")
                nc.vector.reciprocal(rec[:], dsum[:])
                nc.vector.tensor_tensor(
                    out=eb_h, in0=eb_h,
                    in1=rec[:].to_broadcast([P, 4, ncc]),
                    op=ALU.mult)
                # G *= alpha (in place, bf16)
                g4 = G3[:, :, 0:P].rearrange("p c (f j) -> p c f j", j=HID)
                nc.vector.tensor_tensor(
                    out=g4, in0=g4,
                    in1=eb_c.to_broadcast([P, ncc, 4, HID]),
                    op=ALU.mult)
                # aggregate: psr[:, s*P:(s+1)*P][feat, slot] += sum_c G[slot, c, feat]
                for c in range(ncc):
                    nc.tensor.matmul(out=psr[:, s * P:(s + 1) * P],
                                     lhsT=G3[:, c, 0:P], rhs=identb_sb[:],
                                     start=False, stop=(c == ncc - 1))

            # ---- phase C/D interleaved per 512-node group
            for gi, gn in groups:
                GW = gn * P
                g0 = gi * 4 * P
                # res1 projection (feature-major) into psr
                psr = pb.tile([P, GW], F32, tag="pbig")
                for k in range(2):
                    nc.tensor.matmul(out=psr[:], lhsT=res1w_sb[:, k * P:(k + 1) * P],
                                     rhs=xe_res[:, k * NPC + g0:k * NPC + g0 + GW],
                                     start=(k == 0), stop=False)
                for s in range(gn):
                    gat1_tile(gi * 4 + s, psr, s)
                # h2 = relu(g1 + res1 + b)
                h2t = wk.tile([P, GW], BF16, tag="h2t")
                nc.scalar.activation(h2t[:], psr[:], AF.Relu, bias=b12_sb[:, 0:1])
                # z/b_src/b_dst projections + res2 (all feature-major)
                psz = pz.tile([42, GW], F32, tag="pzo")
                nc.tensor.matmul(out=psz[:], lhsT=l2_sb[:], rhs=h2t[:],
                                 start=True, stop=True)
                z_sd = wk.tile([42, GW], BF16, tag="zsd")
                nc.vector.tensor_copy(z_sd[:], psz[:])
                pso = pz.tile([OUT, GW], F32, tag="pzo")
                nc.tensor.matmul(out=pso[:], lhsT=r2w_sb[:], rhs=h2t[:],
                                 start=True, stop=True)
                nc.vector.tensor_copy(res2F[:, g0:g0 + GW], pso[:])
                for s in range(gn):
                    ti = gi * 4 + s
                    ptrz = ptp.tile([P, 42], BF16, tag="ptr")
                    nc.tensor.transpose(ptrz[:], z_sd[:, s * P:(s + 1) * P],
                                        identb_sb[0:42, 0:42])
                    tb2 = tbp.tile([P, TB2], BF16, tag="tb2")
                    nc.vector.tensor_copy(tb2[:, 0:41], ptrz[:, 0:41])
                    nc.vector.tensor_copy(bdst_nm[:, ti:ti + 1], ptrz[:, 41:42])
                    nc.sync.dma_start(slice2[ti * P:(ti + 1) * P, :], tb2[:])

            # ---- AllGather table2
            nc.gpsimd.collective_compute(
                "AllGather", ALU.bypass, replica_groups=[list(range(NCORE))],
                ins=[slice2[:]], outs=[full2[:]])

            # ---- phase E: GAT2 aggregation
            for ti in range(NTILE):
                nlo, nhi = NLO[ti], NHI[ti]
                ncc = nlo + nhi
                G2 = gp2.tile([P, ncc * TB2], BF16, tag="G2")
                split_gather(G2, ti, TB2, full2)
                G23 = G2[:].rearrange("p (c e) -> p c e", e=TB2)
                e2 = wk.tile([P, ncc], F32, tag="e2")
                nc.vector.tensor_tensor(
                    out=e2[:], in0=G23[:, :, 40:41].rearrange("p c f -> p (c f)"),
                    in1=bdst_nm[:, ti:ti + 1].to_broadcast([P, ncc]), op=ALU.add)
                e2tmp = wk.tile([P, ncc], F32, tag="e2tmp")
                nc.vector.tensor_scalar_mul(e2tmp[:], e2[:], SLOPE)
                nc.vector.tensor_tensor(out=e2[:], in0=e2[:], in1=e2tmp[:],
                                        op=ALU.max)
                nc.scalar.activation(e2[:], e2[:], AF.Exp)
                msl = mask_sb[:, CSTART[ti]:CSTART[ti] + ncc]
                nc.vector.tensor_tensor(out=e2[:], in0=e2[:], in1=msl, op=ALU.mult)
                dsum2 = wk.tile([P, 1], F32, tag="dsum2")
                nc.vector.tensor_reduce(dsum2[:], e2[:], AX.X, ALU.add)
                nc.vector.tensor_scalar_max(dsum2[:], dsum2[:], 1e-16)
                rec2 = wk.tile([P, 1], F32, tag="rec2")
                nc.vector.reciprocal(rec2[:], dsum2[:])
                nc.vector.tensor_tensor(out=e2[:], in0=e2[:],
                                        in1=rec2[:].to_broadcast([P, ncc]),
                                        op=ALU.mult)
                nc.vector.tensor_tensor(
                    out=G23[:, :, 0:OUT], in0=G23[:, :, 0:OUT],
                    in1=e2[:].to_broadcast([P, ncc, OUT]),
                    op=ALU.mult)
                po = ptp.tile([OUT, P], F32, tag="ptr")
                for c in range(ncc):
                    nc.tensor.matmul(out=po[:], lhsT=G23[:, c, 0:OUT],
                                     rhs=identb_sb[:],
                                     start=(c == 0), stop=(c == ncc - 1))
                ot = wk.tile([OUT, P], F32, tag="ot")
                nc.vector.scalar_tensor_tensor(
                    out=ot[:], in0=po[:], scalar=b2c_sb[0:OUT, 0:1],
                    in1=res2F[:, ti * P:(ti + 1) * P],
                    op0=ALU.add, op1=ALU.add)
                nc.sync.dma_start(out_d[:, ti * P:(ti + 1) * P], ot[:])

    nc.finalize()
    return nc


def kernel(**inputs):
    meta, in_maps, old_ids = _prepare(inputs)
    key = meta["key"]
    if key not in _cache:
        _cache[key] = _build(meta)
    nc = _cache[key]
    res = bass_utils.run_bass_kernel_spmd(nc, in_maps, core_ids=list(range(NCORE)))
    outp = np.zeros((NPAD, OUT), np.float32)
    for c in range(NCORE):
        outp[old_ids[c]] = res.results[c]["out"].T
    return outp[:N]



# revision 25
# speedup vs baseline: 1.3113x; 1.3113x over previous
"""GAT with autoencoder + residuals on 8 trn2 NeuronCores (Bass/Tile).

v3 "pair-gather": nodes are renumbered by in-degree (desc) and dealt
round-robin to the 8 cores; each dst node owns one SBUF partition of its
128-node tile, its non-self incoming edges occupy chunk columns of that
partition.  The projection tables are gathered as PAIRS of consecutive
rows (1KB / 512B elements, idx = table_row >> 1), so a single int16 index
window covers all 50176 rows -- no lo/hi split, near-ideal slot packing
(CTOT = max-degree bound).  The wrong half of each pair is killed by a
-1e30 bias before exp (alpha -> 0).  Self loops never touch DMA: they are
applied from resident local tables via diag(alpha_self) matmuls.
Aggregation is unnormalized (node-major psum, lhsT=I stationary), the
softmax reciprocal is folded in afterwards on the vector engine, and a
final identity-matmul transpose drops the result feature-major onto the
residual-projection PSUM.  All dense math in bf16; two AllGathers publish
the per-core tables between phases.
"""
import hashlib
import numpy as np
import ml_dtypes

import concourse.bacc as bacc
import concourse.mybir as mybir
import concourse.tile as tile
from concourse import bass_utils

# model sizes (fixed by the problem)
N = 50000
IN = 512
ENC = 256
HID = 32
HEADS = 4
OUT = 40
SLOPE = 0.2

NCORE = 8
P = 128
NTILE = 49
NPC = NTILE * P          # 6272 nodes per core
NPAD = NCORE * NPC       # 50176
NEG = -1.0e30

F32 = mybir.dt.float32
BF16 = mybir.dt.bfloat16
I16 = mybir.dt.int16
AF = mybir.ActivationFunctionType
ALU = mybir.AluOpType
AX = mybir.AxisListType

TB1 = 192    # bf16 cols per table1 row (384B): h 0:128 | a_src f32 64:68 | pad
TB2 = 64     # bf16 cols per table2 row (128B): z 0:40 | b_src 40 | pad
SUBCH1 = 8   # pair chunks per sub-gather (HW cap: 64 descriptors/engine)
SUBCH2 = 8
GELEM = SUBCH1 * 2 * TB1  # bf16 elements of a G buffer

_cache = {}


def _wrap_idx(blk):
    """[128, NB] slot-major block -> dma_gather idx layout [128, 8*NB] int16."""
    nb = blk.shape[1]
    flat = blk.T.reshape(-1)                 # j = c*128 + p
    w = flat.reshape(-1, 16).T               # [16, 8*NB]
    return np.tile(w, (8, 1)).astype(np.int16)


def _prepare(inputs):
    x = np.asarray(inputs["x"], np.float32)
    ei = np.asarray(inputs["edge_index"]).astype(np.int64)
    src0, dst0 = ei[0], ei[1]

    deg = np.bincount(np.concatenate([dst0, np.arange(N)]), minlength=NPAD)
    order = np.argsort(-deg, kind="stable")
    rank = np.empty(NPAD, np.int64)
    rank[order] = np.arange(NPAD)
    tidx_of = (rank % NCORE) * NPC + rank // NCORE   # table row of each node

    er = rank[dst0]           # dst rank per non-self edge
    est = tidx_of[src0]       # src table row per edge
    B = NCORE * P
    deg_ns = np.bincount(er, minlength=NPAD)
    NC = np.array([int(deg_ns[t * B:(t + 1) * B].max()) for t in range(NTILE)])
    CSTART = np.zeros(NTILE, np.int64)
    CSTART[1:] = np.cumsum(NC)[:-1]
    CTOT = int(NC.sum())

    # slot assignment: j-th edge of a node -> column j of its tile run
    eord = np.argsort(er, kind="stable")
    er_s = er[eord]
    est_s = est[eord]
    boundaries = np.flatnonzero(np.r_[True, er_s[1:] != er_s[:-1]])
    counts = np.diff(np.r_[boundaries, er_s.size])
    j = np.arange(er_s.size) - np.repeat(boundaries, counts)
    et_s = er_s // B
    epart_s = (er_s // NCORE) % P
    ecore_s = er_s % NCORE
    colg = CSTART[et_s] + j

    sval = np.zeros((NCORE, P, CTOT), np.int16)
    bias2 = np.full((NCORE, P, 2 * CTOT), NEG, np.float32)
    sval[ecore_s, epart_s, colg] = (est_s >> 1).astype(np.int16)
    half = (est_s & 1).astype(np.int64)
    bias2[ecore_s, epart_s, 2 * colg + half] = 0.0

    WSTART = np.zeros(NTILE, np.int64)
    w_off = 0
    for t in range(NTILE):
        WSTART[t] = w_off
        w_off += 8 * int(NC[t])
    WTOT = int(w_off)
    idx_all = np.zeros((NCORE, P, WTOT), np.int16)
    for c in range(NCORE):
        for t in range(NTILE):
            cs = int(CSTART[t])
            idx_all[c][:, int(WSTART[t]):int(WSTART[t]) + 8 * int(NC[t])] = \
                _wrap_idx(sval[c][:, cs:cs + int(NC[t])])

    # per-core x (transposed, feature-major, bf16)
    xp = np.zeros((NPAD, IN), np.float32)
    xp[:N] = x
    old_ids = np.empty((NCORE, NPC), np.int64)
    x_t = np.empty((NCORE, IN, NPC), ml_dtypes.bfloat16)
    for c in range(NCORE):
        ids = order[c + NCORE * np.arange(NPC)]
        old_ids[c] = ids
        x_t[c] = xp[ids].T.astype(ml_dtypes.bfloat16)

    # replicated derived weights
    w = lambda k: np.asarray(inputs[k], np.float32)
    bf = lambda a: np.ascontiguousarray(a).astype(ml_dtypes.bfloat16)
    a1sd = np.zeros((P, 8), np.float32)
    for h in range(HEADS):
        a1sd[h * HID:(h + 1) * HID, h] = w("gat1_att_src")[h]
        a1sd[h * HID:(h + 1) * HID, 4 + h] = w("gat1_att_dst")[h]
    vs2 = w("gat2_w") @ w("gat2_att_src")[0]
    vd2 = w("gat2_w") @ w("gat2_att_dst")[0]
    lhsT2 = np.concatenate([w("gat2_w"), vs2[:, None], vd2[:, None]], 1)  # [128,42]
    bias12 = (w("gat1_b") + w("res1_b")).reshape(P, 1)
    b2c = np.zeros((P, 1), np.float32)
    b2c[:OUT, 0] = w("gat2_b") + w("res2_b")

    consts = {
        "identb": np.eye(P, dtype=np.float32).astype(ml_dtypes.bfloat16),
        "ident": np.eye(P, dtype=np.float32),
        "w1": bf(w("ae_w1")), "b1p": w("ae_b1").reshape(2, P).T.copy(),
        "w2": bf(w("ae_w2")), "b2p": w("ae_b2").reshape(2, P).T.copy(),
        "g1w": bf(w("gat1_w")), "a1sd": bf(a1sd),
        "res1w": bf(w("res1_w")), "b12p": bias12,
        "l2p": bf(lhsT2), "r2w": bf(w("res2_w")), "b2cp": b2c,
    }
    meta = {
        "NC": NC.tolist(), "CSTART": CSTART.tolist(), "CTOT": CTOT,
        "WSTART": WSTART.tolist(), "WTOT": WTOT,
        "key": "v3:" + hashlib.sha1(ei.tobytes()).hexdigest(),
    }
    in_maps = []
    for c in range(NCORE):
        m = {"x_t": x_t[c], "idx_all": idx_all[c], "bias_all": bias2[c]}
        m.update(consts)
        in_maps.append(m)
    return meta, in_maps, old_ids


def _build(meta):
    NC = meta["NC"]
    CSTART, CTOT = meta["CSTART"], meta["CTOT"]
    WSTART, WTOT = meta["WSTART"], meta["WTOT"]

    nc = bacc.Bacc("TRN2", target_bir_lowering=False, debug=False,
                   num_devices=NCORE, num_swdge_queues=4)
    # I/O
    x_t = nc.dram_tensor("x_t", [IN, NPC], BF16, kind="ExternalInput")
    idx_all = nc.dram_tensor("idx_all", [P, WTOT], I16, kind="ExternalInput")
    bias_all = nc.dram_tensor("bias_all", [P, 2 * CTOT], F32, kind="ExternalInput")
    identb = nc.dram_tensor("identb", [P, P], BF16, kind="ExternalInput")
    ident = nc.dram_tensor("ident", [P, P], F32, kind="ExternalInput")
    w1 = nc.dram_tensor("w1", [IN, ENC], BF16, kind="ExternalInput")
    b1p = nc.dram_tensor("b1p", [P, 2], F32, kind="ExternalInput")
    w2 = nc.dram_tensor("w2", [ENC, ENC], BF16, kind="ExternalInput")
    b2p = nc.dram_tensor("b2p", [P, 2], F32, kind="ExternalInput")
    g1w = nc.dram_tensor("g1w", [ENC, P], BF16, kind="ExternalInput")
    a1sd = nc.dram_tensor("a1sd", [P, 8], BF16, kind="ExternalInput")
    res1w = nc.dram_tensor("res1w", [ENC, P], BF16, kind="ExternalInput")
    b12p = nc.dram_tensor("b12p", [P, 1], F32, kind="ExternalInput")
    l2p = nc.dram_tensor("l2p", [P, 42], BF16, kind="ExternalInput")
    r2w = nc.dram_tensor("r2w", [P, OUT], BF16, kind="ExternalInput")
    b2cp = nc.dram_tensor("b2cp", [P, 1], F32, kind="ExternalInput")
    out_d = nc.dram_tensor("out", [OUT, NPC], F32, kind="ExternalOutput")

    groups = [(gi, min(4, NTILE - gi * 4)) for gi in range((NTILE + 3) // 4)]

    with tile.TileContext(nc) as tc:
        with (
            tc.tile_pool(name="const", bufs=1) as cp,
            tc.tile_pool(name="pers", bufs=1) as pp,
            tc.tile_pool(name="dram", bufs=1, space="DRAM") as dp,
            tc.tile_pool(name="xk", bufs=2) as xkp,
            tc.tile_pool(name="wk", bufs=2) as wk,
            tc.tile_pool(name="tb", bufs=3) as tbp,
            tc.tile_pool(name="gg", bufs=3) as gp,
            tc.tile_pool(name="pbig", bufs=2, space="PSUM") as pb,
            tc.tile_pool(name="pzo", bufs=2, space="PSUM") as pz,
            tc.tile_pool(name="pagg", bufs=2, space="PSUM") as pu,
            tc.tile_pool(name="ptr", bufs=2, space="PSUM") as ptp,
        ):
            # ---- load constants
            identb_sb = cp.tile([P, P], BF16)
            nc.sync.dma_start(identb_sb[:], identb[:])
            ident_sb = cp.tile([P, P], F32)
            nc.sync.dma_start(ident_sb[:], ident[:])
            w1_sb = cp.tile([P, 4 * ENC], BF16)
            nc.sync.dma_start(
                w1_sb[:].rearrange("p (k m) -> p k m", m=ENC),
                w1[:].rearrange("(k p) m -> p k m", p=P))
            w2_sb = cp.tile([P, 2 * ENC], BF16)
            nc.sync.dma_start(
                w2_sb[:].rearrange("p (k m) -> p k m", m=ENC),
                w2[:].rearrange("(k p) m -> p k m", p=P))
            g1w_sb = cp.tile([P, 2 * P], BF16)
            nc.sync.dma_start(
                g1w_sb[:].rearrange("p (k m) -> p k m", m=P),
                g1w[:].rearrange("(k p) m -> p k m", p=P))
            res1w_sb = cp.tile([P, 2 * P], BF16)
            nc.sync.dma_start(
                res1w_sb[:].rearrange("p (k m) -> p k m", m=P),
                res1w[:].rearrange("(k p) m -> p k m", p=P))
            b1_sb = cp.tile([P, 2], F32)
            nc.sync.dma_start(b1_sb[:], b1p[:])
            b2_sb = cp.tile([P, 2], F32)
            nc.sync.dma_start(b2_sb[:], b2p[:])
            a1sd_sb = cp.tile([P, 8], BF16)
            nc.sync.dma_start(a1sd_sb[:], a1sd[:])
            b12_sb = cp.tile([P, 1], F32)
            nc.sync.dma_start(b12_sb[:], b12p[:])
            l2_sb = cp.tile([P, 42], BF16)
            nc.sync.dma_start(l2_sb[:], l2p[:])
            r2w_sb = cp.tile([P, OUT], BF16)
            nc.sync.dma_start(r2w_sb[:], r2w[:])
            b2c_sb = cp.tile([P, 1], F32)
            nc.sync.dma_start(b2c_sb[:], b2cp[:])
            idx_sb = pp.tile([P, WTOT], I16)
            nc.sync.dma_start(idx_sb[:], idx_all[:])
            bias_sb = pp.tile([P, 2 * CTOT], F32)
            nc.sync.dma_start(bias_sb[:], bias_all[:])

            adst_nm = pp.tile([P, NTILE * 4], F32)
            asrc_nm = pp.tile([P, NTILE * 4], F32)
            bdst_nm = pp.tile([P, NTILE], F32)
            aself = pp.tile([P, NTILE * 4], F32)    # exp(lrelu(asrc+adst))
            a2self = pp.tile([P, NTILE], F32)
            tb1loc = pp.tile([P, NTILE * P], BF16)  # transposed h per tile
            tb2loc = pp.tile([P, NTILE * 41], BF16)  # transposed z|b_src per tile
            xe_res = pp.tile([P, 2 * NPC], BF16)    # encoder output, resident
            res2F = pp.tile([OUT, NPC], F32)        # res2 branch, feature-major

            # internal DRAM
            slice1 = dp.tile([NPC, TB1], BF16)
            full1 = dp.tile([NPAD, TB1], BF16, addr_space="Shared")
            slice2 = dp.tile([NPC, TB2], BF16)
            full2 = dp.tile([NPAD, TB2], BF16, addr_space="Shared")



            # ---- phase A+B: autoencoder, GAT1 projections, table1 rows
            for gi, gn in groups:
                GW = gn * P
                g0 = gi * 4 * P
                xks = []
                for k in range(4):
                    xk = xkp.tile([P, GW], BF16, tag=f"xk{k}")
                    nc.sync.dma_start(xk[:], x_t[k * P:(k + 1) * P, g0:g0 + GW])
                    xks.append(xk)
                z1s = []
                for m in range(2):
                    ps1 = pb.tile([P, GW], F32, tag="pbig")
                    for k in range(4):
                        nc.tensor.matmul(
                            out=ps1[:], lhsT=w1_sb[:, k * ENC + m * P:k * ENC + (m + 1) * P],
                            rhs=xks[k][:], start=(k == 0), stop=(k == 3))
                    z1 = wk.tile([P, GW], BF16, tag=f"z1{m}")
                    nc.scalar.activation(z1[:], ps1[:], AF.Relu, bias=b1_sb[:, m:m + 1])
                    z1s.append(z1)
                for m in range(2):
                    ps2 = pb.tile([P, GW], F32, tag="pbig")
                    for k in range(2):
                        nc.tensor.matmul(
                            out=ps2[:], lhsT=w2_sb[:, k * ENC + m * P:k * ENC + (m + 1) * P],
                            rhs=z1s[k][:], start=(k == 0), stop=(k == 1))
                    nc.scalar.activation(xe_res[:, m * NPC + g0:m * NPC + g0 + GW],
                                         ps2[:], AF.Relu, bias=b2_sb[:, m:m + 1])
                # h = xe @ gat1_w ; a_src/a_dst
                psh = pb.tile([P, GW], F32, tag="pbig")
                for k in range(2):
                    nc.tensor.matmul(out=psh[:], lhsT=g1w_sb[:, k * P:(k + 1) * P],
                                     rhs=xe_res[:, k * NPC + g0:k * NPC + g0 + GW],
                                     start=(k == 0), stop=(k == 1))
                h_sb = wk.tile([P, GW], BF16, tag="hsb")
                nc.vector.tensor_copy(h_sb[:], psh[:])
                psa = pz.tile([8, GW], F32, tag="pzo")
                nc.tensor.matmul(out=psa[:], lhsT=a1sd_sb[:], rhs=h_sb[:],
                                 start=True, stop=True)
                a_sd = wk.tile([8, GW], F32, tag="asd")
                nc.vector.tensor_copy(a_sd[:], psa[:])
                for s in range(gn):
                    ti = gi * 4 + s
                    ptr1 = ptp.tile([P, P], BF16, tag="ptr")
                    nc.tensor.transpose(ptr1[:], h_sb[:, s * P:(s + 1) * P], identb_sb[:])
                    tb1 = tbp.tile([P, TB1], BF16, tag="tb1")
                    nc.vector.memset(tb1[:, 136:TB1], 0.0)
                    nc.vector.tensor_copy(tb1[:, 0:P], ptr1[:])
                    nc.vector.tensor_copy(tb1loc[:, ti * P:(ti + 1) * P], ptr1[:])
                    ptr2 = ptp.tile([P, 8], F32, tag="ptr")
                    nc.tensor.transpose(ptr2[:], a_sd[:, s * P:(s + 1) * P], ident_sb[0:8, 0:8])
                    f32v = tb1[:].bitcast(F32)
                    nc.vector.tensor_copy(f32v[:, 64:68], ptr2[:, 0:4])
                    nc.vector.tensor_copy(asrc_nm[:, ti * 4:(ti + 1) * 4], ptr2[:, 0:4])
                    nc.vector.tensor_copy(adst_nm[:, ti * 4:(ti + 1) * 4], ptr2[:, 4:8])
                    nc.sync.dma_start(slice1[ti * P:(ti + 1) * P, :], tb1[:])

            # aself = exp(leakyrelu(asrc + adst))
            nc.vector.tensor_tensor(out=aself[:], in0=asrc_nm[:], in1=adst_nm[:],
                                    op=ALU.add)
            stmp = wk.tile([P, NTILE * 4], F32, tag="stmp")
            nc.vector.tensor_scalar_mul(stmp[:], aself[:], SLOPE)
            nc.vector.tensor_tensor(out=aself[:], in0=aself[:], in1=stmp[:],
                                    op=ALU.max)
            nc.scalar.activation(aself[:], aself[:], AF.Exp)

            # ---- AllGather table1
            nc.gpsimd.collective_compute(
                "AllGather", ALU.bypass, replica_groups=[list(range(NCORE))],
                ins=[slice1[:]], outs=[full1[:]])

            full1p = full1[:].rearrange("(j k) e -> j (k e)", k=2)  # [25088, 512]
            full2p = full2[:].rearrange("(j k) e -> j (k e)", k=2)  # [25088, 256]
            qrot = [0]

            def gat1_tile(ti, psr, s, last):
                """Gathers + attention + unnormalized aggregation for one dst
                tile; folds softmax reciprocal and adds the result
                feature-major onto psr[:, s*P:(s+1)*P]."""
                cs = CSTART[ti]
                ncc = NC[ti]
                dsum = wk.tile([P, 4], F32, tag="dsum")
                nc.vector.tensor_copy(dsum[:], aself[:, ti * 4:(ti + 1) * 4])
                psu = pu.tile([P, P], F32, tag="psu")
                first = True
                a = 0
                while a < ncc:
                    b = min(a + SUBCH1, ncc)
                    nb = b - a
                    G = gp.tile([P, GELEM], BF16, tag="G")
                    G3 = G[:].rearrange("p (c e) -> p c e", e=2 * TB1)
                    nc.gpsimd.dma_gather(
                        G3[:, 0:nb, :], full1p,
                        idx_sb[:, WSTART[ti] + 8 * a:WSTART[ti] + 8 * b],
                        128 * nb, 128 * nb, 2 * TB1,
                        queue_num=qrot[0] % 4, single_packet=True)
                    qrot[0] += 1
                    Gf = G[:].bitcast(F32).rearrange("p (c f) -> p c f", f=TB1)
                    # e = a_src + a_dst + side-bias ; rows 0:4 lo, 4:8 hi
                    eb = wk.tile([P, 8 * SUBCH1], F32, tag="eb")
                    ebh = eb[:, 0:8 * nb].rearrange("p (f c) -> p f c", f=8)
                    adsl = adst_nm[:, ti * 4:(ti + 1) * 4]
                    nc.vector.tensor_tensor(
                        out=ebh[:, 0:4, :],
                        in0=Gf[:, 0:nb, 64:68].rearrange("p c f -> p f c"),
                        in1=adsl.to_broadcast([P, 4, nb]), op=ALU.add)
                    nc.vector.tensor_tensor(
                        out=ebh[:, 4:8, :],
                        in0=Gf[:, 0:nb, TB1 // 2 + 64:TB1 // 2 + 68].rearrange(
                            "p c f -> p f c"),
                        in1=adsl.to_broadcast([P, 4, nb]), op=ALU.add)
                    ebc = eb[:, 0:8 * nb].rearrange("p (f c) -> p c f", c=nb)
                    bv = bias_sb[:, 2 * (cs + a):2 * (cs + b)].rearrange(
                        "p (c k) -> p c k", k=2)
                    nc.vector.tensor_tensor(
                        out=ebc[:, :, 0:4], in0=ebc[:, :, 0:4],
                        in1=bv[:, :, 0:1].to_broadcast([P, nb, 4]), op=ALU.add)
                    nc.vector.tensor_tensor(
                        out=ebc[:, :, 4:8], in0=ebc[:, :, 4:8],
                        in1=bv[:, :, 1:2].to_broadcast([P, nb, 4]), op=ALU.add)
                    et = wk.tile([P, 8 * SUBCH1], F32, tag="et")
                    nc.vector.tensor_scalar_mul(et[:, 0:8 * nb], eb[:, 0:8 * nb], SLOPE)
                    nc.vector.tensor_tensor(out=eb[:, 0:8 * nb], in0=eb[:, 0:8 * nb],
                                            in1=et[:, 0:8 * nb], op=ALU.max)
                    nc.scalar.activation(eb[:, 0:8 * nb], eb[:, 0:8 * nb], AF.Exp)
                    red = wk.tile([P, 8], F32, tag="red")
                    nc.vector.tensor_reduce(red[:], ebh, AX.X, ALU.add)
                    nc.vector.tensor_tensor(out=dsum[:], in0=dsum[:],
                                            in1=red[:, 0:4], op=ALU.add)
                    nc.vector.tensor_tensor(out=dsum[:], in0=dsum[:],
                                            in1=red[:, 4:8], op=ALU.add)
                    # weight the gathered halves
                    g4lo = G3[:, 0:nb, 0:P].rearrange("p c (f j) -> p c f j", j=HID)
                    nc.vector.tensor_tensor(
                        out=g4lo, in0=g4lo,
                        in1=ebc[:, :, 0:4].to_broadcast([P, nb, 4, HID]), op=ALU.mult)
                    g4hi = G3[:, 0:nb, TB1:TB1 + P].rearrange("p c (f j) -> p c f j", j=HID)
                    nc.vector.tensor_tensor(
                        out=g4hi, in0=g4hi,
                        in1=ebc[:, :, 4:8].to_broadcast([P, nb, 4, HID]), op=ALU.mult)
                    # node-major accumulation, lhsT = I stationary
                    for c in range(nb):
                        nc.tensor.matmul(out=psu[:], lhsT=identb_sb[:],
                                         rhs=G3[:, c, 0:P], start=first, stop=False)
                        first = False
                        nc.tensor.matmul(out=psu[:], lhsT=identb_sb[:],
                                         rhs=G3[:, c, TB1:TB1 + P], start=False, stop=False)
                    a = b
                # self loop from the local transposed table
                for h in range(HEADS):
                    dg = wk.tile([P, P], BF16, tag="diag")
                    nc.vector.tensor_tensor(
                        out=dg[:], in0=identb_sb[:],
                        in1=aself[:, ti * 4 + h:ti * 4 + h + 1].to_broadcast([P, P]),
                        op=ALU.mult)
                    nc.tensor.matmul(
                        out=psu[:, h * HID:(h + 1) * HID], lhsT=dg[:],
                        rhs=tb1loc[:, ti * P + h * HID:ti * P + (h + 1) * HID],
                        start=False, stop=(h == HEADS - 1))
                # fold reciprocal, transpose onto the residual psum
                nc.vector.tensor_scalar_max(dsum[:], dsum[:], 1e-16)
                rec = wk.tile([P, 4], F32, tag="rec")
                nc.vector.reciprocal(rec[:], dsum[:])
                aggN = wk.tile([P, P], BF16, tag="aggN")
                nc.vector.tensor_tensor(
                    out=aggN[:].rearrange("p (f j) -> p f j", f=4),
                    in0=psu[:].rearrange("p (f j) -> p f j", f=4),
                    in1=rec[:].to_broadcast([P, 4, HID]), op=ALU.mult)
                nc.tensor.matmul(out=psr[:, s * P:(s + 1) * P], lhsT=aggN[:],
                                 rhs=identb_sb[:], start=False, stop=last)

            # ---- phase C/D interleaved per 512-node group
            for gi, gn in groups:
                GW = gn * P
                g0 = gi * 4 * P
                psr = pb.tile([P, GW], F32, tag="pbig")
                for k in range(2):
                    nc.tensor.matmul(out=psr[:], lhsT=res1w_sb[:, k * P:(k + 1) * P],
                                     rhs=xe_res[:, k * NPC + g0:k * NPC + g0 + GW],
                                     start=(k == 0), stop=False)
                for s in range(gn):
                    gat1_tile(gi * 4 + s, psr, s, s == gn - 1)
                # h2 = relu(g1 + res1 + b)
                h2t = wk.tile([P, GW], BF16, tag="h2t")
                nc.scalar.activation(h2t[:], psr[:], AF.Relu, bias=b12_sb[:, 0:1])
                # z/b_src/b_dst projections + res2 (all feature-major)
                psz = pz.tile([42, GW], F32, tag="pzo")
                nc.tensor.matmul(out=psz[:], lhsT=l2_sb[:], rhs=h2t[:],
                                 start=True, stop=True)
                z_sd = wk.tile([42, GW], BF16, tag="zsd")
                nc.vector.tensor_copy(z_sd[:], psz[:])
                pso = pz.tile([OUT, GW], F32, tag="pzo")
                nc.tensor.matmul(out=pso[:], lhsT=r2w_sb[:], rhs=h2t[:],
                                 start=True, stop=True)
                nc.vector.tensor_copy(res2F[:, g0:g0 + GW], pso[:])
                for s in range(gn):
                    ti = gi * 4 + s
                    ptrz = ptp.tile([P, 42], BF16, tag="ptr")
                    nc.tensor.transpose(ptrz[:], z_sd[:, s * P:(s + 1) * P],
                                        identb_sb[0:42, 0:42])
                    tb2 = tbp.tile([P, TB2], BF16, tag="tb2")
                    nc.vector.memset(tb2[:, 41:TB2], 0.0)
                    nc.vector.tensor_copy(tb2[:, 0:41], ptrz[:, 0:41])
                    nc.vector.tensor_copy(tb2loc[:, ti * 41:(ti + 1) * 41], ptrz[:, 0:41])
                    nc.vector.tensor_copy(bdst_nm[:, ti:ti + 1], ptrz[:, 41:42])
                    nc.sync.dma_start(slice2[ti * P:(ti + 1) * P, :], tb2[:])

            # a2self = exp(leakyrelu(b_src + b_dst))
            bsrc_v = tb2loc[:].rearrange("p (t c) -> p t c", c=41)[:, :, 40:41]
            nc.vector.tensor_tensor(out=a2self[:].rearrange("p (t c) -> p t c", c=1),
                                    in0=bsrc_v, in1=bdst_nm[:].rearrange(
                                        "p (t c) -> p t c", c=1), op=ALU.add)
            stmp2 = wk.tile([P, NTILE], F32, tag="stmp2")
            nc.vector.tensor_scalar_mul(stmp2[:], a2self[:], SLOPE)
            nc.vector.tensor_tensor(out=a2self[:], in0=a2self[:], in1=stmp2[:],
                                    op=ALU.max)
            nc.scalar.activation(a2self[:], a2self[:], AF.Exp)

            # ---- AllGather table2
            nc.gpsimd.collective_compute(
                "AllGather", ALU.bypass, replica_groups=[list(range(NCORE))],
                ins=[slice2[:]], outs=[full2[:]])

            # ---- phase E: GAT2
            for ti in range(NTILE):
                cs = CSTART[ti]
                ncc = NC[ti]
                den2 = wk.tile([P, 1], F32, tag="den2")
                nc.vector.tensor_copy(den2[:], a2self[:, ti:ti + 1])
                psu2 = pu.tile([P, OUT], F32, tag="psu")
                first = True
                a = 0
                while a < ncc:
                    b = min(a + SUBCH2, ncc)
                    nb = b - a
                    G = gp.tile([P, GELEM], BF16, tag="G")
                    G23 = G[:].rearrange("p (c e) -> p c e", e=2 * TB2)
                    nc.gpsimd.dma_gather(
                        G23[:, 0:nb, :], full2p,
                        idx_sb[:, WSTART[ti] + 8 * a:WSTART[ti] + 8 * b],
                        128 * nb, 128 * nb, 2 * TB2,
                        queue_num=qrot[0] % 4, single_packet=True)
                    qrot[0] += 1
                    e2 = wk.tile([P, 2 * SUBCH2], F32, tag="e2")
                    e2h = e2[:, 0:2 * nb].rearrange("p (k c) -> p k c", k=2)
                    bds = bdst_nm[:, ti:ti + 1]
                    nc.vector.tensor_tensor(
                        out=e2h[:, 0:1, :],
                        in0=G23[:, 0:nb, 40:41].rearrange("p c f -> p f c"),
                        in1=bds.to_broadcast([P, 1, nb]), op=ALU.add)
                    nc.vector.tensor_tensor(
                        out=e2h[:, 1:2, :],
                        in0=G23[:, 0:nb, TB2 + 40:TB2 + 41].rearrange("p c f -> p f c"),
                        in1=bds.to_broadcast([P, 1, nb]), op=ALU.add)
                    e2c = e2[:, 0:2 * nb].rearrange("p (k c) -> p c k", c=nb)
                    nc.vector.tensor_tensor(
                        out=e2c, in0=e2c,
                        in1=bias_sb[:, 2 * (cs + a):2 * (cs + b)].rearrange(
                            "p (c k) -> p c k", k=2), op=ALU.add)
                    et2 = wk.tile([P, 2 * SUBCH2], F32, tag="et2")
                    nc.vector.tensor_scalar_mul(et2[:, 0:2 * nb], e2[:, 0:2 * nb], SLOPE)
                    nc.vector.tensor_tensor(out=e2[:, 0:2 * nb], in0=e2[:, 0:2 * nb],
                                            in1=et2[:, 0:2 * nb], op=ALU.max)
                    nc.scalar.activation(e2[:, 0:2 * nb], e2[:, 0:2 * nb], AF.Exp)
                    red2 = wk.tile([P, 1], F32, tag="red2")
                    nc.vector.tensor_reduce(red2[:], e2[:, 0:2 * nb], AX.X, ALU.add)
                    nc.vector.tensor_tensor(out=den2[:], in0=den2[:], in1=red2[:],
                                            op=ALU.add)
                    zlo = G23[:, 0:nb, 0:OUT]
                    nc.vector.tensor_tensor(
                        out=zlo, in0=zlo,
                        in1=e2[:, 0:nb].to_broadcast([P, nb, OUT]), op=ALU.mult)
                    zhi = G23[:, 0:nb, TB2:TB2 + OUT]
                    nc.vector.tensor_tensor(
                        out=zhi, in0=zhi,
                        in1=e2[:, nb:2 * nb].to_broadcast([P, nb, OUT]), op=ALU.mult)
                    for c in range(nb):
                        nc.tensor.matmul(out=psu2[:], lhsT=identb_sb[:],
                                         rhs=G23[:, c, 0:OUT], start=first, stop=False)
                        first = False
                        nc.tensor.matmul(out=psu2[:], lhsT=identb_sb[:],
                                         rhs=G23[:, c, TB2:TB2 + OUT],
                                         start=False, stop=False)
                    a = b
                dg2 = wk.tile([P, P], BF16, tag="diag")
                nc.vector.tensor_tensor(
                    out=dg2[:], in0=identb_sb[:],
                    in1=a2self[:, ti:ti + 1].to_broadcast([P, P]), op=ALU.mult)
                nc.tensor.matmul(out=psu2[:], lhsT=dg2[:],
                                 rhs=tb2loc[:, ti * 41:ti * 41 + OUT],
                                 start=False, stop=True)
                nc.vector.tensor_scalar_max(den2[:], den2[:], 1e-16)
                rec2 = wk.tile([P, 1], F32, tag="rec2")
                nc.vector.reciprocal(rec2[:], den2[:])
                aggN2 = wk.tile([P, OUT], BF16, tag="aggN2")
                nc.vector.tensor_tensor(out=aggN2[:], in0=psu2[:],
                                        in1=rec2[:].to_broadcast([P, OUT]), op=ALU.mult)
                po = pz.tile([OUT, P], F32, tag="pzo")
                nc.tensor.matmul(out=po[:], lhsT=aggN2[:], rhs=identb_sb[:],
                                 start=True, stop=True)
                ot = wk.tile([OUT, P], F32, tag="ot")
                nc.vector.scalar_tensor_tensor(
                    out=ot[:], in0=po[:], scalar=b2c_sb[0:OUT, 0:1],
                    in1=res2F[:, ti * P:(ti + 1) * P],
                    op0=ALU.add, op1=ALU.add)
                nc.sync.dma_start(out_d[:, ti * P:(ti + 1) * P], ot[:])

    nc.finalize()
    return nc


def kernel(**inputs):
    meta, in_maps, old_ids = _prepare(inputs)
    key = meta["key"]
    if key not in _cache:
        _cache[key] = _build(meta)
    nc = _cache[key]
    res = bass_utils.run_bass_kernel_spmd(nc, in_maps, core_ids=list(range(NCORE)))
    outp = np.zeros((NPAD, OUT), np.float32)
    for c in range(NCORE):
        outp[old_ids[c]] = res.results[c]["out"].T
    return outp[:N]
